# revision 8
# baseline (speedup 1.0000x reference)
"""Trainium2 Bass kernel for nn_MixtureModel (moe_routing).

Data-parallel over the 64 sequences: each of the 8 NeuronCores handles 8
sequences (1024 batch rows) and all K=8 mixture components; per-component
MLP weights are replicated.

Per-core pipeline (single Bass launch):
  phase 1: per (k, b-chunk): MM1 in feature-major layout (h_T[hh,b], bias
           via per-partition ACT Relu), MM2 batch-major (logits[b,a]) with
           h_T slices as the stationary operand.  Each logits PSUM chunk is
           evicted once through ScalarE Exp (with a per-k constant shift)
           into an fp16 SBUF cache, with accum_out giving Z; a one-hot
           iota gather on VectorE extracts the action logit.
  recursion: cumsum over time via tensor_tensor_scan + two small PE
           transposes + logsumexp over K with 3D-AP reductions.
  phase 2: Sum_k exp(c_k) * exp_k accumulated on the PE with diagonal
           stationary matrices read from the fp16 cache; Ln on ScalarE.

The host computes final_mixture_logprobs from the exported per-sequence
cumulative action logprobs (a [64,8] log_softmax - negligible work).
"""

import numpy as np
import ml_dtypes

import concourse.bass as bass
import concourse.mybir as mybir
import concourse.tile as tile
from concourse import masks
from concourse.bass_utils import run_bass_kernel_spmd

F32 = mybir.dt.float32
F16 = mybir.dt.float16
AL = mybir.AluOpType
AF = mybir.ActivationFunctionType
AX = mybir.AxisListType

K, D, H, A = 8, 512, 512, 1024
S, T = 64, 128
B = S * T
N_CORES = 8
S_LOC = S // N_CORES          # sequences per core
B_LOC = B // N_CORES          # batch rows per core
NSUB = B_LOC // 128           # 128-row batch subtiles per core (== S_LOC)
DT = D // 128                 # contraction tiles for MM1
HT = H // 128                 # contraction tiles for MM2
NBC = B_LOC // 512            # 512-wide b-chunks for MM1
NAC = A // 512                # 512-wide a-chunks for MM2

MAT_DT = F16                  # matmul operand dtype (fp16: 1 cyc/row on PE)


def _split_multiwaits(nc):
    """This toolchain's walrus rejects instructions carrying more than one
    sync wait.  Move extra waits onto no-op instructions inserted just
    before the owner on the same engine (engines execute in order, so all
    waits still complete before the instruction runs)."""
    uid = 0
    for f in nc.m.functions:
        for b in f.blocks:
            il = b.instructions
            if not any(
                ins.sync_info is not None and len(ins.sync_info.on_wait) > 1
                for ins in il
            ):
                continue
            out = []
            for ins in il:
                si = ins.sync_info
                if si is not None and len(si.on_wait) > 1:
                    waits = list(si.on_wait)
                    for w in waits[:-1]:
                        uid += 1
                        out.append(mybir.InstNoOp(
                            name=f"{ins.name}-sw{uid}",
                            sync_info=mybir.SyncInfo(on_wait=[w], on_update=[]),
                            bass_nofuse=True,
                            engine=ins.engine,
                        ))
                    ins.sync_info = mybir.SyncInfo(
                        on_wait=[waits[-1]], on_update=list(si.on_update)
                    )
                out.append(ins)
            b.instructions = out


def _build_program(use_b2: bool, split: bool = True):
    nc = bass.Bass("TRN2", target_bir_lowering=False, debug=False)

    obsT = nc.dram_tensor("obsT", [D, B_LOC], MAT_DT, kind="ExternalInput").ap()
    w1 = nc.dram_tensor("w1", [K, D, H], MAT_DT, kind="ExternalInput").ap()
    w2 = nc.dram_tensor("w2", [K, H, A], MAT_DT, kind="ExternalInput").ap()
    b1c = nc.dram_tensor("b1c", [128, K * HT], F32, kind="ExternalInput").ap()
    ck = nc.dram_tensor("ck", [128, K], F32, kind="ExternalInput").ap()       # -C_k
    ckcols = nc.dram_tensor("ckcols", [128, NSUB * K], F32, kind="ExternalInput").ap()
    actf = nc.dram_tensor("actf", [128, NSUB], F32, kind="ExternalInput").ap()
    prior = nc.dram_tensor("prior", [S_LOC * K, 1], F32, kind="ExternalInput").ap()
    if use_b2:
        b2r = nc.dram_tensor("b2r", [1, K * A], MAT_DT, kind="ExternalInput").ap()

    mo = nc.dram_tensor("mo", [B_LOC, A], F32, kind="ExternalOutput").ap()
    cl = nc.dram_tensor("cl", [S_LOC * K, 1], F32, kind="ExternalOutput").ap()

    with tile.TileContext(nc) as tc:
        with (
            tc.tile_pool(name="const", bufs=1) as constp,
            tc.tile_pool(name="obst", bufs=1) as obstp,
            tc.tile_pool(name="w1p", bufs=2) as w1p,
            tc.tile_pool(name="w2p", bufs=2) as w2p,
            tc.tile_pool(name="hp", bufs=2) as hp,
            tc.tile_pool(name="expc", bufs=1) as expcp,
            tc.tile_pool(name="small", bufs=1) as sp,
            tc.tile_pool(name="pp16", bufs=2) as pp16,
            tc.tile_pool(name="outp", bufs=2) as outp,
            tc.tile_pool(name="psA", bufs=2, space="PSUM") as psA,
            tc.tile_pool(name="psB", bufs=2, space="PSUM") as psB,
            tc.tile_pool(name="psT", bufs=1, space="PSUM") as psT,
        ):
            # ---- constants / small inputs ----
            ident = constp.tile([128, 128], F32)
            masks.make_identity(nc, ident[:])
            iota_i = constp.tile([128, A], mybir.dt.int32)
            nc.gpsimd.iota(iota_i[:], [[1, A]], base=0, channel_multiplier=0)
            iota_f = constp.tile([128, A], F32)
            nc.vector.tensor_copy(iota_f[:], iota_i[:])

            b1sb = sp.tile([128, K * HT], F32)
            nc.sync.dma_start(b1sb[:], b1c[:])
            cksb = sp.tile([128, K], F32)
            nc.sync.dma_start(cksb[:], ck[:])
            ckcsb = sp.tile([128, NSUB * K], F32)
            nc.sync.dma_start(ckcsb[:], ckcols[:])
            actsb = sp.tile([128, NSUB], F32)
            nc.sync.dma_start(actsb[:], actf[:])
            priorsb = sp.tile([S_LOC * K, 1], F32)
            nc.sync.dma_start(priorsb[:], prior[:])
            if use_b2:
                b2sb = sp.tile([1, K * A], MAT_DT)
                nc.sync.dma_start(b2sb[:], b2r[:])
                ones1 = constp.tile([1, 128], MAT_DT)
                nc.vector.memset(ones1[:], 1.0)

            obst = obstp.tile([128, DT, B_LOC], MAT_DT)
            obsT_r = obsT.rearrange("(dt p) b -> p dt b", p=128)
            for dt_ in range(DT):
                nc.sync.dma_start(obst[:, dt_, :], obsT_r[:, dt_, :])

            zeros64 = constp.tile([S_LOC * K, T], F32)
            nc.vector.memset(zeros64[:], 0.0)

            # accumulators written per (k, bsub)
            zacc = sp.tile([128, NSUB * K], F32)
            gacc = sp.tile([128, NSUB * K], F32)
            trash = sp.tile([128, A], F32)

            # fp16 exp cache: col = bsub*(K*A) + k*A + a
            expc = expcp.tile([128, NSUB * K * A], F16)

            # ---- phase 1 ----
            for k in range(K):
                w1t = w1p.tile([128, DT, H], MAT_DT)
                nc.sync.dma_start(w1t[:], w1[k].rearrange("(dt p) h -> p dt h", p=128))
                w2t = w2p.tile([128, HT, A], MAT_DT)
                nc.sync.dma_start(w2t[:], w2[k].rearrange("(ht p) a -> p ht a", p=128))

                for bc in range(NBC):
                    # MM1: h_T[hh, b] for this 512-wide b-chunk
                    ht_t = hp.tile([128, HT, 512], MAT_DT)
                    for ht in range(HT):
                        ps = psA.tile([128, 512], F32)
                        for dt_ in range(DT):
                            nc.tensor.matmul(
                                ps[:],
                                lhsT=w1t[:, dt_, ht * 128:(ht + 1) * 128],
                                rhs=obst[:, dt_, bc * 512:(bc + 1) * 512],
                                start=(dt_ == 0),
                                stop=(dt_ == DT - 1),
                            )
                        nc.scalar.activation(
                            ht_t[:, ht, :], ps[:], AF.Relu,
                            bias=b1sb[:, k * HT + ht: k * HT + ht + 1], scale=1.0,
                        )

                    # MM2 + exp eviction + gather, per 128-row batch subtile
                    for bs4 in range(4):
                        bsub = bc * 4 + bs4
                        ps2 = psB.tile([128, A], F32)
                        for ac in range(NAC):
                            sl = ps2[:, ac * 512:(ac + 1) * 512]
                            if use_b2:
                                nc.tensor.matmul(
                                    sl, lhsT=ones1[:],
                                    rhs=b2sb[0:1, k * A + ac * 512: k * A + (ac + 1) * 512],
                                    start=True, stop=False,
                                )
                            for ht in range(HT):
                                nc.tensor.matmul(
                                    sl,
                                    lhsT=ht_t[:, ht, bs4 * 128:(bs4 + 1) * 128],
                                    rhs=w2t[:, ht, ac * 512:(ac + 1) * 512],
                                    start=(ht == 0 and not use_b2),
                                    stop=(ht == HT - 1),
                                )
                        col = bsub * K + k
                        nc.scalar.activation(
                            expc[:, bsub * (K * A) + k * A:
                                    bsub * (K * A) + (k + 1) * A],
                            ps2[:], AF.Exp,
                            bias=cksb[:, k:k + 1], scale=1.0,
                            accum_out=zacc[:, col:col + 1],
                        )
                        nc.vector.scalar_tensor_tensor(
                            trash[:], iota_f[:],
                            actsb[:, bsub:bsub + 1], ps2[:],
                            AL.is_equal, AL.mult,
                            accum_out=gacc[:, col:col + 1],
                        )

            # ---- combine pass-1 stats ----
            lnz = sp.tile([128, NSUB * K], F32)
            nc.scalar.activation(lnz[:], zacc[:], AF.Ln)
            # alp = L + (-C_k) - lnZ   (col = bsub*K + k)
            alp = sp.tile([128, NSUB * K], F32)
            t0 = sp.tile([128, NSUB * K], F32)
            nc.vector.tensor_tensor(t0[:], gacc[:], ckcsb[:], AL.add)
            nc.vector.tensor_tensor(alp[:], t0[:], lnz[:], AL.subtract)

            # ---- mixture recursion ----
            alpT_ps = psT.tile([NSUB * K, 128], F32)
            nc.tensor.transpose(alpT_ps[:], alp[:], ident[:])
            alpT = sp.tile([NSUB * K, T], F32)
            nc.vector.tensor_copy(alpT[:], alpT_ps[:])
            cum = sp.tile([NSUB * K, T], F32)
            nc.vector.tensor_tensor_scan(
                cum[:], alpT[:], zeros64[:], 0.0, AL.add, AL.add
            )
            nc.sync.dma_start(cl[:], cum[:, T - 1:T])
            shift = sp.tile([NSUB * K, T], F32)
            nc.vector.tensor_tensor(shift[:], cum[:], alpT[:], AL.subtract)
            z64 = sp.tile([NSUB * K, T], F32)
            nc.vector.tensor_scalar_add(z64[:], shift[:], priorsb[:])

            zT_ps = psT.tile([128, NSUB * K], F32)
            nc.tensor.transpose(zT_ps[:], z64[:], ident[0:NSUB * K, 0:NSUB * K])
            zT = sp.tile([128, NSUB * K], F32)
            nc.vector.tensor_copy(zT[:], zT_ps[:])

            zT3 = zT[:].rearrange("p (s k) -> p s k", k=K)
            m8 = sp.tile([128, NSUB], F32)
            nc.vector.tensor_reduce(m8[:], zT3, AX.X, AL.max, negate=True)
            d64 = sp.tile([128, NSUB * K], F32)
            nc.vector.tensor_tensor(
                d64[:].rearrange("p (s k) -> p s k", k=K),
                zT3, m8[:].unsqueeze(2).broadcast_to([128, NSUB, K]), AL.add
            )
            e64 = sp.tile([128, NSUB * K], F32)
            nc.scalar.activation(e64[:], d64[:], AF.Exp)
            z8 = sp.tile([128, NSUB], F32)
            nc.vector.tensor_reduce(
                z8[:], e64[:].rearrange("p (s k) -> p s k", k=K), AX.X, AL.add
            )
            lnz8 = sp.tile([128, NSUB], F32)
            nc.scalar.activation(lnz8[:], z8[:], AF.Ln)
            mix = sp.tile([128, NSUB * K], F32)
            nc.vector.tensor_tensor(
                mix[:].rearrange("p (s k) -> p s k", k=K),
                d64[:].rearrange("p (s k) -> p s k", k=K),
                lnz8[:].unsqueeze(2).broadcast_to([128, NSUB, K]), AL.subtract
            )
            c64 = sp.tile([128, NSUB * K], F32)
            nc.vector.tensor_tensor(c64[:], mix[:], lnz[:], AL.subtract)
            sc64 = sp.tile([128, NSUB * K], F32)
            nc.scalar.activation(sc64[:], c64[:], AF.Exp)

            # ---- phase 2: p = sum_k sc_k * exp_k  (fp16 ping-pong on DVE) ----
            for bsub in range(NSUB):
                p_prev = None
                for k in range(K):
                    col = bsub * K + k
                    esl = expc[:, bsub * (K * A) + k * A: bsub * (K * A) + (k + 1) * A]
                    p_cur = pp16.tile([128, A], F16, tag="p")
                    if p_prev is None:
                        nc.vector.tensor_scalar_mul(
                            p_cur[:], esl, sc64[:, col:col + 1]
                        )
                    else:
                        nc.vector.scalar_tensor_tensor(
                            p_cur[:], esl, sc64[:, col:col + 1], p_prev[:],
                            AL.mult, AL.add,
                        )
                    p_prev = p_cur
                ob = outp.tile([128, A], F32)
                nc.scalar.activation(ob[:], p_prev[:], AF.Ln)
                nc.sync.dma_start(mo[bsub * 128:(bsub + 1) * 128, :], ob[:])

    if split:
        _split_multiwaits(nc)
    return nc


_PROG_CACHE = {}


def _get_program(use_b2: bool):
    if use_b2 not in _PROG_CACHE:
        _PROG_CACHE[use_b2] = _build_program(use_b2)
    return _PROG_CACHE[use_b2]


def _prepare(obs, actions, start, W1, b1, W2, b2):
    use_b2 = bool(np.any(b2))

    # per-k constant shift C_k: probe a few rows so exp(L - C_k) stays in
    # a safe fp16/fp32 range for any input scale close to the probe's.
    rows = obs[:: max(1, B // 64)][:64]
    hP = np.maximum(rows @ W1 + b1[:, None, :], 0.0)           # [K, P, H]
    lP = np.einsum("kph,kha->kpa", hP, W2) + b2[:, None, :]     # [K, P, A]
    Ck = lP.max(axis=(1, 2)).astype(np.float32) + 2.0           # [K]

    w1h = W1.astype(np.float16)
    w2h = W2.astype(np.float16)

    # b1 per-partition bias columns: col = k*HT + ht, partition = hh % 128
    b1c = np.ascontiguousarray(
        b1.reshape(K, HT, 128).transpose(2, 0, 1).reshape(128, K * HT)
    ).astype(np.float32)
    ckrep = np.broadcast_to(-Ck[None, :], (128, K)).astype(np.float32)
    ckcols = np.ascontiguousarray(np.broadcast_to(
        -Ck[None, None, :], (128, NSUB, K)
    ).reshape(128, NSUB * K)).astype(np.float32)

    in_maps = []
    for c in range(N_CORES):
        bsl = slice(c * B_LOC, (c + 1) * B_LOC)
        obs_c = obs[bsl]
        act_c = actions[bsl].astype(np.int64)
        m = {
            "obsT": np.ascontiguousarray(obs_c.T).astype(np.float16),
            "w1": w1h,
            "w2": w2h,
            "b1c": b1c,
            "ck": ckrep,
            "ckcols": ckcols,
            "actf": np.ascontiguousarray(
                act_c.reshape(NSUB, 128).T.astype(np.float32)
            ),
            "prior": np.ascontiguousarray(
                start[c * S_LOC:(c + 1) * S_LOC].reshape(S_LOC * K, 1)
            ).astype(np.float32),
        }
        if use_b2:
            m["b2r"] = b2.reshape(1, K * A).astype(np.float16)
        in_maps.append(m)
    return use_b2, in_maps


def _postprocess(results, start):
    model_out = np.concatenate(
        [results[c]["mo"] for c in range(N_CORES)], axis=0
    ).astype(np.float32)
    cumlast = np.concatenate(
        [results[c]["cl"].reshape(S_LOC, K) for c in range(N_CORES)], axis=0
    )
    zf = start + cumlast
    zf = zf - zf.max(axis=1, keepdims=True)
    final = zf - np.log(np.exp(zf).sum(axis=1, keepdims=True))
    return model_out, final.astype(np.float32)


def kernel(obs, actions, start_mixture_logprobs, W1, b1, W2, b2, seq_len):
    obs = np.asarray(obs, np.float32)
    actions = np.asarray(actions)
    start = np.asarray(start_mixture_logprobs, np.float32)
    W1 = np.asarray(W1, np.float32)
    b1 = np.asarray(b1, np.float32)
    W2 = np.asarray(W2, np.float32)
    b2 = np.asarray(b2, np.float32)

    use_b2, in_maps = _prepare(obs, actions, start, W1, b1, W2, b2)
    nc = _get_program(use_b2)
    res = run_bass_kernel_spmd(nc, in_maps, list(range(N_CORES)))
    return _postprocess(res.results, start)


# revision 10
# speedup vs baseline: 1.2995x; 1.2995x over previous
"""Trainium2 Bass kernel for nn_MixtureModel (moe_routing).

Data-parallel over the 64 sequences: each of the 8 NeuronCores handles 8
sequences (1024 batch rows) and all K=8 mixture components; per-component
MLP weights are replicated.

Per-core pipeline (single Bass launch):
  phase 1: per (k, b-chunk): MM1 in feature-major layout (h_T[hh,b], bias
           via per-partition ACT Relu), MM2 batch-major (logits[b,a]) with
           h_T slices as the stationary operand.  Each logits PSUM chunk is
           evicted once through ScalarE Exp (with a per-k constant shift)
           into an fp16 SBUF cache, with accum_out giving Z; a one-hot
           iota gather on VectorE extracts the action logit.
  recursion: cumsum over time via tensor_tensor_scan + two small PE
           transposes + logsumexp over K with 3D-AP reductions.
  phase 2: Sum_k exp(c_k) * exp_k accumulated on the PE with diagonal
           stationary matrices read from the fp16 cache; Ln on ScalarE.

The host computes final_mixture_logprobs from the exported per-sequence
cumulative action logprobs (a [64,8] log_softmax - negligible work).
"""

import numpy as np
import ml_dtypes

import concourse.bass as bass
import concourse.mybir as mybir
import concourse.tile as tile
from concourse import masks
from concourse.bass_utils import run_bass_kernel_spmd

F32 = mybir.dt.float32
F16 = mybir.dt.float16
AL = mybir.AluOpType
AF = mybir.ActivationFunctionType
AX = mybir.AxisListType

K, D, H, A = 8, 512, 512, 1024
S, T = 64, 128
B = S * T
N_CORES = 8
S_LOC = S // N_CORES          # sequences per core
B_LOC = B // N_CORES          # batch rows per core
NSUB = B_LOC // 128           # 128-row batch subtiles per core (== S_LOC)
DT = D // 128                 # contraction tiles for MM1
HT = H // 128                 # contraction tiles for MM2
NBC = B_LOC // 512            # 512-wide b-chunks for MM1
NAC = A // 512                # 512-wide a-chunks for MM2

MAT_DT = F16                  # matmul operand dtype (fp16: 1 cyc/row on PE)


def _split_multiwaits(nc):
    """This toolchain's walrus rejects instructions carrying more than one
    sync wait.  Move extra waits onto no-op instructions inserted just
    before the owner on the same engine (engines execute in order, so all
    waits still complete before the instruction runs)."""
    uid = 0
    for f in nc.m.functions:
        for b in f.blocks:
            il = b.instructions
            if not any(
                ins.sync_info is not None and len(ins.sync_info.on_wait) > 1
                for ins in il
            ):
                continue
            out = []
            for ins in il:
                si = ins.sync_info
                if si is not None and len(si.on_wait) > 1:
                    waits = list(si.on_wait)
                    for w in waits[:-1]:
                        uid += 1
                        out.append(mybir.InstNoOp(
                            name=f"{ins.name}-sw{uid}",
                            sync_info=mybir.SyncInfo(on_wait=[w], on_update=[]),
                            bass_nofuse=True,
                            engine=ins.engine,
                        ))
                    ins.sync_info = mybir.SyncInfo(
                        on_wait=[waits[-1]], on_update=list(si.on_update)
                    )
                out.append(ins)
            b.instructions = out


def _build_program(use_b2: bool, split: bool = True):
    nc = bass.Bass("TRN2", target_bir_lowering=False, debug=False)

    obsT = nc.dram_tensor("obsT", [D, B_LOC], MAT_DT, kind="ExternalInput").ap()
    w1 = nc.dram_tensor("w1", [K, D, H], MAT_DT, kind="ExternalInput").ap()
    w2 = nc.dram_tensor("w2", [K, H, A], MAT_DT, kind="ExternalInput").ap()
    b1c = nc.dram_tensor("b1c", [128, K * HT], F32, kind="ExternalInput").ap()
    ck = nc.dram_tensor("ck", [128, K], F32, kind="ExternalInput").ap()       # -C_k
    ckcols = nc.dram_tensor("ckcols", [128, NSUB * K], F32, kind="ExternalInput").ap()
    actf = nc.dram_tensor("actf", [128, NSUB], F32, kind="ExternalInput").ap()
    prior = nc.dram_tensor("prior", [S_LOC * K, 1], F32, kind="ExternalInput").ap()
    if use_b2:
        b2r = nc.dram_tensor("b2r", [1, K * A], MAT_DT, kind="ExternalInput").ap()

    mo = nc.dram_tensor("mo", [B_LOC, A], F32, kind="ExternalOutput").ap()
    cl = nc.dram_tensor("cl", [S_LOC * K, 1], F32, kind="ExternalOutput").ap()

    from contextlib import ExitStack
    with tile.TileContext(nc) as tc:
        with (
            tc.tile_pool(name="const", bufs=1) as constp,
            tc.tile_pool(name="obst", bufs=1) as obstp,
            tc.tile_pool(name="w1p", bufs=2) as w1p,
            tc.tile_pool(name="w2p", bufs=2) as w2p,
            tc.tile_pool(name="hp", bufs=2) as hp,
            tc.tile_pool(name="expc", bufs=1) as expcp,
            tc.tile_pool(name="small", bufs=1) as sp,
            tc.tile_pool(name="diagp", bufs=16) as diagp,
            tc.tile_pool(name="outp", bufs=2) as outp,
            tc.tile_pool(name="psA", bufs=2, space="PSUM") as psA,
            tc.tile_pool(name="psT", bufs=1, space="PSUM") as psT,
        ):
            psB_ctx = ExitStack()
            psB = psB_ctx.enter_context(tc.tile_pool(name="psB", bufs=5, space="PSUM"))
            # ---- load obsT and k=0 weights first (feeds the first matmuls) ----
            obst = obstp.tile([128, DT, B_LOC], MAT_DT)
            obsT_r = obsT.rearrange("(dt p) b -> p dt b", p=128)
            for dt_ in range(DT):
                for bc in range(NBC):
                    nc.sync.dma_start(
                        obst[:, dt_, bc * 512:(bc + 1) * 512],
                        obsT_r[:, dt_, bc * 512:(bc + 1) * 512],
                    )

            # ---- constants / small inputs ----
            ident = constp.tile([128, 128], F32)
            masks.make_identity(nc, ident[:])
            iota_i = constp.tile([128, A], mybir.dt.int32)
            nc.gpsimd.iota(iota_i[:], [[1, A]], base=0, channel_multiplier=0)
            iota_f = constp.tile([128, A], F32)
            nc.vector.tensor_copy(iota_f[:], iota_i[:])
            iotd_i = constp.tile([128, 128], mybir.dt.int32)
            nc.gpsimd.iota(iotd_i[:], [[1, 128]], base=0, channel_multiplier=-1)
            iotd_f = constp.tile([128, 128], F32)
            nc.vector.tensor_copy(iotd_f[:], iotd_i[:])

            b1sb = sp.tile([128, K * HT], F32)
            nc.sync.dma_start(b1sb[:], b1c[:])
            cksb = sp.tile([128, K], F32)
            nc.sync.dma_start(cksb[:], ck[:])
            ckcsb = sp.tile([128, NSUB * K], F32)
            nc.sync.dma_start(ckcsb[:], ckcols[:])
            actsb = sp.tile([128, NSUB], F32)
            nc.sync.dma_start(actsb[:], actf[:])
            priorsb = sp.tile([S_LOC * K, 1], F32)
            nc.sync.dma_start(priorsb[:], prior[:])
            if use_b2:
                b2sb = sp.tile([1, K * A], MAT_DT)
                nc.sync.dma_start(b2sb[:], b2r[:])
                ones1 = constp.tile([1, 128], MAT_DT)
                nc.vector.memset(ones1[:], 1.0)

            zeros64 = constp.tile([S_LOC * K, T], F32)
            nc.vector.memset(zeros64[:], 0.0)

            # accumulators written per (k, bsub, achunk)
            zacc = sp.tile([128, NSUB * K * NAC], F32)
            gacc = sp.tile([128, NSUB * K * NAC], F32)
            trash = sp.tile([128, 512], F32)

            # fp16 exp cache: col = bsub*(K*A) + k*A + a
            expc = expcp.tile([128, NSUB * K * A], F16)

            # ---- phase 1 ----
            for k in range(K):
                w1t = w1p.tile([128, DT, H], MAT_DT)
                nc.sync.dma_start(w1t[:], w1[k].rearrange("(dt p) h -> p dt h", p=128))
                w2t = w2p.tile([128, HT, A], MAT_DT)
                nc.sync.dma_start(w2t[:], w2[k].rearrange("(ht p) a -> p ht a", p=128))

                for bc in range(NBC):
                    # MM1: h_T[hh, b] for this 512-wide b-chunk
                    ht_t = hp.tile([128, HT, 512], MAT_DT)
                    for ht in range(HT):
                        ps = psA.tile([128, 512], F32)
                        for dt_ in range(DT):
                            nc.tensor.matmul(
                                ps[:],
                                lhsT=w1t[:, dt_, ht * 128:(ht + 1) * 128],
                                rhs=obst[:, dt_, bc * 512:(bc + 1) * 512],
                                start=(dt_ == 0),
                                stop=(dt_ == DT - 1),
                            )
                        nc.scalar.activation(
                            ht_t[:, ht, :], ps[:], AF.Relu,
                            bias=b1sb[:, k * HT + ht: k * HT + ht + 1], scale=1.0,
                        )

                    # MM2 + exp eviction + gather, per 128-row batch subtile
                    for bs4 in range(4):
                        bsub = bc * 4 + bs4
                        for ac in range(NAC):
                            ps2 = psB.tile([128, 512], F32)
                            if use_b2:
                                nc.tensor.matmul(
                                    ps2[:], lhsT=ones1[:],
                                    rhs=b2sb[0:1, k * A + ac * 512: k * A + (ac + 1) * 512],
                                    start=True, stop=False,
                                )
                            for ht in range(HT):
                                nc.tensor.matmul(
                                    ps2[:],
                                    lhsT=ht_t[:, ht, bs4 * 128:(bs4 + 1) * 128],
                                    rhs=w2t[:, ht, ac * 512:(ac + 1) * 512],
                                    start=(ht == 0 and not use_b2),
                                    stop=(ht == HT - 1),
                                )
                            col = (bsub * K + k) * NAC + ac
                            nc.scalar.activation(
                                expc[:, bsub * (K * A) + k * A + ac * 512:
                                        bsub * (K * A) + k * A + (ac + 1) * 512],
                                ps2[:], AF.Exp,
                                bias=cksb[:, k:k + 1], scale=1.0,
                                accum_out=zacc[:, col:col + 1],
                            )
                            nc.vector.scalar_tensor_tensor(
                                trash[:], iota_f[:, ac * 512:(ac + 1) * 512],
                                actsb[:, bsub:bsub + 1], ps2[:],
                                AL.is_equal, AL.mult,
                                accum_out=gacc[:, col:col + 1],
                            )

            # ---- combine pass-1 stats ----
            psB_ctx.close()
            zsum = sp.tile([128, NSUB * K], F32)
            z2 = zacc[:].rearrange("p (c two) -> p c two", two=NAC)
            nc.vector.tensor_tensor(zsum[:], z2[:, :, 0], z2[:, :, 1], AL.add)
            gsum = sp.tile([128, NSUB * K], F32)
            g2 = gacc[:].rearrange("p (c two) -> p c two", two=NAC)
            nc.vector.tensor_tensor(gsum[:], g2[:, :, 0], g2[:, :, 1], AL.add)
            lnz = sp.tile([128, NSUB * K], F32)
            nc.scalar.activation(lnz[:], zsum[:], AF.Ln)
            # alp = L + (-C_k) - lnZ   (col = bsub*K + k)
            alp = sp.tile([128, NSUB * K], F32)
            t0 = sp.tile([128, NSUB * K], F32)
            nc.vector.tensor_tensor(t0[:], gsum[:], ckcsb[:], AL.add)
            nc.vector.tensor_tensor(alp[:], t0[:], lnz[:], AL.subtract)

            # ---- mixture recursion ----
            alpT_ps = psT.tile([NSUB * K, 128], F32, tag="tp")
            nc.tensor.transpose(alpT_ps[:], alp[:], ident[:])
            alpT = sp.tile([NSUB * K, T], F32)
            nc.vector.tensor_copy(alpT[:], alpT_ps[:])
            cum = sp.tile([NSUB * K, T], F32)
            nc.vector.tensor_tensor_scan(
                cum[:], alpT[:], zeros64[:], 0.0, AL.add, AL.add
            )
            nc.sync.dma_start(cl[:], cum[:, T - 1:T])
            shift = sp.tile([NSUB * K, T], F32)
            nc.vector.tensor_tensor(shift[:], cum[:], alpT[:], AL.subtract)
            z64 = sp.tile([NSUB * K, T], F32)
            nc.vector.tensor_scalar_add(z64[:], shift[:], priorsb[:])

            zT_ps = psT.tile([128, NSUB * K], F32, tag="tp")
            nc.tensor.transpose(zT_ps[:], z64[:], ident[0:NSUB * K, 0:NSUB * K])
            zT = sp.tile([128, NSUB * K], F32)
            nc.vector.tensor_copy(zT[:], zT_ps[:])

            zT3 = zT[:].rearrange("p (s k) -> p s k", k=K)
            m8 = sp.tile([128, NSUB], F32)
            nc.vector.tensor_reduce(m8[:], zT3, AX.X, AL.max, negate=True)
            d64 = sp.tile([128, NSUB * K], F32)
            nc.vector.tensor_tensor(
                d64[:].rearrange("p (s k) -> p s k", k=K),
                zT3, m8[:].unsqueeze(2).broadcast_to([128, NSUB, K]), AL.add
            )
            e64 = sp.tile([128, NSUB * K], F32)
            nc.scalar.activation(e64[:], d64[:], AF.Exp)
            z8 = sp.tile([128, NSUB], F32)
            nc.vector.tensor_reduce(
                z8[:], e64[:].rearrange("p (s k) -> p s k", k=K), AX.X, AL.add
            )
            lnz8 = sp.tile([128, NSUB], F32)
            nc.scalar.activation(lnz8[:], z8[:], AF.Ln)
            mix = sp.tile([128, NSUB * K], F32)
            nc.vector.tensor_tensor(
                mix[:].rearrange("p (s k) -> p s k", k=K),
                d64[:].rearrange("p (s k) -> p s k", k=K),
                lnz8[:].unsqueeze(2).broadcast_to([128, NSUB, K]), AL.subtract
            )
            c64 = sp.tile([128, NSUB * K], F32)
            nc.vector.tensor_tensor(c64[:], mix[:], lnz[:], AL.subtract)
            sc64 = sp.tile([128, NSUB * K], F32)
            nc.scalar.activation(sc64[:], c64[:], AF.Exp)

            # ---- phase 2: p = sum_k sc * exp  via diagonal matmuls ----
            with tc.tile_pool(name="psP", bufs=4, space="PSUM") as psP:
                for bsub in range(NSUB):
                    dgs = []
                    for k in range(K):
                        dg = diagp.tile([128, 128], F16, tag="dg")
                        col = bsub * K + k
                        nc.vector.scalar_tensor_tensor(
                            dg[:], iotd_f[:], 0.0,
                            sc64[:, col:col + 1].broadcast_to([128, 128]),
                            AL.is_equal, AL.mult,
                        )
                        dgs.append(dg)
                    ob = outp.tile([128, A], F32)
                    for ac in range(NAC):
                        pp = psP.tile([128, 512], F32)
                        for k in range(K):
                            nc.tensor.matmul(
                                pp[:], lhsT=dgs[k][:],
                                rhs=expc[:, bsub * (K * A) + k * A + ac * 512:
                                            bsub * (K * A) + k * A + (ac + 1) * 512],
                                start=(k == 0), stop=(k == K - 1),
                            )
                        nc.scalar.activation(ob[:, ac * 512:(ac + 1) * 512], pp[:], AF.Ln)
                    nc.sync.dma_start(mo[bsub * 128:(bsub + 1) * 128, :], ob[:])

    if split:
        _split_multiwaits(nc)
    return nc


_PROG_CACHE = {}


def _get_program(use_b2: bool):
    if use_b2 not in _PROG_CACHE:
        _PROG_CACHE[use_b2] = _build_program(use_b2)
    return _PROG_CACHE[use_b2]


def _prepare(obs, actions, start, W1, b1, W2, b2):
    use_b2 = bool(np.any(b2))

    # per-k constant shift C_k: probe a few rows so exp(L - C_k) stays in
    # a safe fp16/fp32 range for any input scale close to the probe's.
    rows = obs[:: max(1, B // 64)][:64]
    hP = np.maximum(rows @ W1 + b1[:, None, :], 0.0)           # [K, P, H]
    lP = np.einsum("kph,kha->kpa", hP, W2) + b2[:, None, :]     # [K, P, A]
    Ck = lP.max(axis=(1, 2)).astype(np.float32) + 2.0           # [K]

    w1h = W1.astype(np.float16)
    w2h = W2.astype(np.float16)

    # b1 per-partition bias columns: col = k*HT + ht, partition = hh % 128
    b1c = np.ascontiguousarray(
        b1.reshape(K, HT, 128).transpose(2, 0, 1).reshape(128, K * HT)
    ).astype(np.float32)
    ckrep = np.broadcast_to(-Ck[None, :], (128, K)).astype(np.float32)
    ckcols = np.ascontiguousarray(np.broadcast_to(
        -Ck[None, None, :], (128, NSUB, K)
    ).reshape(128, NSUB * K)).astype(np.float32)

    in_maps = []
    for c in range(N_CORES):
        bsl = slice(c * B_LOC, (c + 1) * B_LOC)
        obs_c = obs[bsl]
        act_c = actions[bsl].astype(np.int64)
        m = {
            "obsT": np.ascontiguousarray(obs_c.T).astype(np.float16),
            "w1": w1h,
            "w2": w2h,
            "b1c": b1c,
            "ck": ckrep,
            "ckcols": ckcols,
            "actf": np.ascontiguousarray(
                act_c.reshape(NSUB, 128).T.astype(np.float32)
            ),
            "prior": np.ascontiguousarray(
                start[c * S_LOC:(c + 1) * S_LOC].reshape(S_LOC * K, 1)
            ).astype(np.float32),
        }
        if use_b2:
            m["b2r"] = b2.reshape(1, K * A).astype(np.float16)
        in_maps.append(m)
    return use_b2, in_maps


def _postprocess(results, start):
    model_out = np.concatenate(
        [results[c]["mo"] for c in range(N_CORES)], axis=0
    ).astype(np.float32)
    cumlast = np.concatenate(
        [results[c]["cl"].reshape(S_LOC, K) for c in range(N_CORES)], axis=0
    )
    zf = start + cumlast
    zf = zf - zf.max(axis=1, keepdims=True)
    final = zf - np.log(np.exp(zf).sum(axis=1, keepdims=True))
    return model_out, final.astype(np.float32)


def kernel(obs, actions, start_mixture_logprobs, W1, b1, W2, b2, seq_len):
    obs = np.asarray(obs, np.float32)
    actions = np.asarray(actions)
    start = np.asarray(start_mixture_logprobs, np.float32)
    W1 = np.asarray(W1, np.float32)
    b1 = np.asarray(b1, np.float32)
    W2 = np.asarray(W2, np.float32)
    b2 = np.asarray(b2, np.float32)

    use_b2, in_maps = _prepare(obs, actions, start, W1, b1, W2, b2)
    nc = _get_program(use_b2)
    res = run_bass_kernel_spmd(nc, in_maps, list(range(N_CORES)))
    return _postprocess(res.results, start)
